# revision 36
# baseline (speedup 1.0000x reference)
"""Trainium2 Bass kernel for the AKT (attention-with-distance-decay) problem.

Reference math (per batch b, head h, dk=32, S=2048, E=256):
    qh, kh, vh = per-head projections of q,k,v
    s  = qh @ kh^T / sqrt(dk)                    (causal-masked)
    p  = softmax(s)                              (softmax #1)
    tail[j] = sum_{j'>j} p[j']                   (1 - cumsum)
    dist = sqrt(clip(tail * (i-j), 0))
    te   = clip(exp(-softplus(gamma_h) * dist), 1e-5, 1e5)
    attn = softmax(where(mask, s*te, -inf))      (softmax #2)
    out  = (attn @ vh)  -> concat heads -> @ Wo^T + bo

Sharding: 8 cores = (batch b = core//2) x (head-group g = core%2, 4 heads each).
Every core runs the identical graph (SPMD); per-core inputs differ.  Each core
emits a partial output (its 4 heads' contribution through Wo); the host adds
the two partials per batch plus bo.

Device-side structure per core (measured 484 us on 8 NeuronCores, l2
rel-err 3.8e-3 vs the fp32 jax reference):
  - host pre-transposes q/k/v to [E, S] bf16 so projections contract over e
    on the partition dim; Wq and bq are pre-scaled by 1/sqrt(dk).
  - qh^T, kh^T stored bf16 [64, 2, S] (head h at partitions (h%2)*32, free
    block h//2 -- PE operands may only start at partition 0/32/64); vh
    stored [S, 4h, 33] bf16 with a ones column so the AV matmul also yields
    the softmax-#2 denominator.
  - causal q-block loop (128 queries, extent = (k+1) key-blocks); the
    diagonal block is masked by accumulating ident^T @ triu(-1e30) onto the
    QK PSUM scores.  Scores are recomputed for softmax #2 rather than held
    in PSUM across the chain (PSUM is the scarce resource; bf16 QK is cheap).
  - softmax #1 skips the max-subtraction (scores are O(5), fp32 exp is
    safe); the key-axis cumsum is a REVERSED tensor_tensor_scan giving the
    exact suffix-sum (no 1-x cancellation); its col 0 is the denominator.
  - decay term in the log domain: u = ln(tail) + ln(pos) + ln(gamma^2)
    - ln(sigma); dist = exp(0.5u); te = exp(-dist).  Ln and Exp share one
    ACT table set (Sqrt cannot coexist with Exp -- mixing them reloads the
    2.7us activation tables every tile); ln(pos) is a reversed AP view of
    one precomputed [128, S] master table; tail=0 or pos=0 give -inf -> te=1
    exactly.  A custom Bacc subclass pins the single covering table set.
  - softmax #2: e2 = exp(max(te,1e-5) * s) directly (no max shift; masked
    lanes are exp(-1e30)=0); Sigma2 rides the vh ones column; the 1/Sigma2
    scale is applied by ACT during the PSUM->SBUF move of the AV output.
  - e2 (bf16) transposed for AV by the DMA xbar (sync engine), one
    3D-output dma_start_transpose per (head, q-block).
  - the whole attention loop is emitted as a 5-stage software pipeline
    (wave = 4 heads of one q-block, consecutive stages run different waves,
    ACT ops first within each stage): engines have in-order queues, so
    naive per-tile emission head-of-line-blocks every engine at ~50% idle.
"""

import os
import sys

for _p in ("/opt/trn_rl_repo", "/root/.axon_site/_ro/trn_rl_repo"):
    if os.path.isdir(_p) and _p not in sys.path:
        sys.path.insert(0, _p)

import ml_dtypes
import numpy as np

import concourse.bacc as bacc
import concourse.bass as bass
import concourse.mybir as mybir
from concourse.tile import TileContext

B, S, E, H = 4, 2048, 256, 8
DK = E // H          # 32
HG = 4               # heads per core
D = HG * DK          # 128, per-core projected width
NCORES = 8

FP = mybir.dt.float32
BF = mybir.dt.bfloat16
AF = mybir.ActivationFunctionType
OP = mybir.AluOpType
NEG = -1e30


class _AktBacc(bacc.Bacc):
    """Bacc whose activation-table placement only considers the one set
    covering every ACT function this kernel uses (Exp, Ln, Identity, Copy).
    The default first-match policy alternates exp_and_others with a
    Ln-capable set, reloading the 2.7us ACT tables per tile."""

    _ACT_SET = "natural_log_exp_and_others"

    def insert_act_table_loads(self):
        import concourse.mybir as _mb
        from concourse.hw_specs import get_activation_tables
        has_activation = any(
            isinstance(i, _mb.InstActivation)
            for b in self.main_func.blocks
            for i in b.instructions
        )
        if not has_activation:
            return
        # positions must stay canonical (act_func_set_id indexes this list)
        tables = [
            (nm, fs if nm == self._ACT_SET else set())
            for nm, fs in get_activation_tables(self.m.arch).items()
        ]
        import bass_rust as _br
        _br.insert_act_table_loads(self, tables)


def build_nc(s_len=S, qk_f32r=False):
    """Build the single-core SPMD graph.  s_len parametrizes the sequence
    length for small-scale simulation tests (must be a multiple of 128)."""
    nqb = s_len // 128           # number of 128-query blocks
    nech = E // 128              # e-chunks (2)

    nc = _AktBacc()
    qT = nc.declare_dram_parameter("qT", [E, s_len], BF, isOutput=False)
    kT = nc.declare_dram_parameter("kT", [E, s_len], BF, isOutput=False)
    vT = nc.declare_dram_parameter("vT", [E, s_len], BF, isOutput=False)
    wqT = nc.declare_dram_parameter("wqT", [E, D], BF, isOutput=False)
    wkT = nc.declare_dram_parameter("wkT", [E, D], BF, isOutput=False)
    wvT = nc.declare_dram_parameter("wvT", [E, D], BF, isOutput=False)
    woT = nc.declare_dram_parameter("woT", [D, E], FP, isOutput=False)
    bqs = nc.declare_dram_parameter("bqs", [64, 2], FP, isOutput=False)
    bks = nc.declare_dram_parameter("bks", [64, 2], FP, isOutput=False)
    bvrow = nc.declare_dram_parameter("bvrow", [1, D], BF, isOutput=False)
    lngsq = nc.declare_dram_parameter("lngsq", [128, HG], FP, isOutput=False)
    out_part = nc.declare_dram_parameter("out_part", [s_len, E], FP, isOutput=True)

    qk_dt = mybir.dt.float32r if qk_f32r else FP

    with TileContext(nc) as tc:
        with (
            tc.tile_pool(name="consts", bufs=1) as consts,
            tc.tile_pool(name="persist", bufs=1) as persist,
        ):
            # ---- constants ----
            ident_f = consts.tile([128, 128], FP)
            nc.vector.memset(ident_f[:], 1.0)
            nc.gpsimd.affine_select(out=ident_f[:], in_=ident_f[:],
                                    compare_op=OP.is_equal, fill=0.0,
                                    base=0, pattern=[[-1, 128]], channel_multiplier=1)
            ident_b = consts.tile([128, 128], BF)
            nc.vector.tensor_copy(out=ident_b[:], in_=ident_f[:])
            # strict upper triangle = NEG, else 0 (diagonal-block causal mask)
            triu_neg = consts.tile([128, 128], BF)
            nc.gpsimd.memset(triu_neg[:], 0.0)
            nc.gpsimd.affine_select(out=triu_neg[:], in_=triu_neg[:],
                                    compare_op=OP.is_ge, fill=NEG,
                                    base=0, pattern=[[-1, 128]], channel_multiplier=1)
            ones1b = consts.tile([1, 128], BF)
            nc.vector.memset(ones1b[:], 1.0)

            lngsq_sb = consts.tile([128, HG], FP)
            nc.sync.dma_start(out=lngsq_sb[:], in_=lngsq[:])
            bq_sb = consts.tile([64, 2], FP)
            nc.sync.dma_start(out=bq_sb[:], in_=bqs[:])
            bk_sb = consts.tile([64, 2], FP)
            nc.sync.dma_start(out=bk_sb[:], in_=bks[:])
            bv_sb = consts.tile([1, D], BF)
            nc.sync.dma_start(out=bv_sb[:], in_=bvrow[:])
            wo_sb = consts.tile([D, E], FP)
            nc.sync.dma_start(out=wo_sb[:], in_=woT[:])

            # master ln(pos) table: lnpos_k[:, j] = M[:, 127 + 128k - j]
            # (a reversed AP view), M[r, c] = ln(r + c - 127), -inf at pos<=0
            lnposM = persist.tile([128, s_len], mybir.dt.float16)


            # ---- persistent activations ----
            # head h lives at partitions (h%2)*32..+32, free-block h//2
            # (PE operands may only start at partition 0/32/64)
            qhT = persist.tile([64, 2, s_len], BF)
            khT = persist.tile([64, 2, s_len], BF)
            vh1 = persist.tile([128, nqb, HG, 33], BF)  # [s-part, s-blk, h, 32d+1]
            nc.vector.memset(vh1[:, :, :, 32:33], 1.0)

            # ---- phase 0: projections ----
            with (
                tc.tile_pool(name="ph0", bufs=2) as ph0,
                tc.tile_pool(name="ph0w", bufs=1) as ph0w,
                tc.tile_pool(name="ph0ps", bufs=2, space="PSUM") as ph0ps,
            ):
                wq_sb = ph0w.tile([128, nech, D], BF)
                wk_sb = ph0w.tile([128, nech, D], BF)
                nc.sync.dma_start(out=wq_sb[:], in_=wqT.rearrange("(c p) d -> p c d", p=128))
                nc.sync.dma_start(out=wk_sb[:], in_=wkT.rearrange("(c p) d -> p c d", p=128))

                for name, src, wsb, bias, dst in (
                    ("q", qT, wq_sb, bq_sb, qhT),
                    ("k", kT, wk_sb, bk_sb, khT),
                ):
                    x_sb = ph0.tile([128, nech, s_len], BF, tag="x_in")
                    nc.sync.dma_start(out=x_sb[:],
                                      in_=src.rearrange("(c p) s -> p c s", p=128))
                    for dg in range(2):          # head-pairs (0,1) and (2,3)
                        for sc in range((s_len + 511) // 512):
                            s0, s1 = sc * 512, min((sc + 1) * 512, s_len)
                            ps = ph0ps.tile([64, 512], FP, tag=f"projps_{name}")
                            for c in range(nech):
                                nc.tensor.matmul(ps[:, 0:s1 - s0],
                                                 lhsT=wsb[:, c, dg * 64:(dg + 1) * 64],
                                                 rhs=x_sb[:, c, s0:s1],
                                                 start=(c == 0), stop=(c == nech - 1))
                            nc.vector.tensor_scalar(
                                out=dst[:, dg, s0:s1], in0=ps[:, 0:s1 - s0],
                                scalar1=bias[:, dg:dg + 1], scalar2=None,
                                op0=OP.add)

                # lnpos master table (scratch freed with this pool)
                lnposM_f = ph0.tile([128, s_len], FP)
                nc.gpsimd.iota(lnposM_f[:], pattern=[[1, s_len]], base=-127,
                               channel_multiplier=1,
                               allow_small_or_imprecise_dtypes=True)
                nc.gpsimd.affine_select(out=lnposM_f[:], in_=lnposM_f[:],
                                        compare_op=OP.is_ge, fill=0.0,
                                        base=-127, pattern=[[1, s_len]],
                                        channel_multiplier=1)
                nc.scalar.activation(out=lnposM[:], in_=lnposM_f[:], func=AF.Ln)

            # ---- attention loop: 5-stage software pipeline ----
            # wave = the 4 heads of one q-block.  Each stage puts its ACT
            # work FIRST and its DVE/PE work after, and consecutive stages
            # run different waves (skew), so no engine queues behind a
            # same-wave dependency on another engine.  Score PSUM tiles are
            # sub-tiled to <=1024 cols (2 banks) so three rotate in 6 banks.
            with (
                tc.tile_pool(name="attv", bufs=1) as attv,
                tc.tile_pool(name="att1", bufs=1) as att1,
                tc.tile_pool(name="att2", bufs=2) as att2,
                tc.tile_pool(name="atte", bufs=4) as atte,
                tc.tile_pool(name="att4", bufs=4) as att4,
                tc.tile_pool(name="ps_s", bufs=3, space="PSUM") as ps_s,
                tc.tile_pool(name="ps_av", bufs=1, space="PSUM") as ps_av,
                tc.tile_pool(name="ps_op", bufs=1, space="PSUM") as ps_op,
            ):
                HF = mybir.dt.float16

                def qk_scores(kq, h, c0, c1):
                    """scores for key-cols [c0, c1) (<=1024 wide) + diagonal
                    mask when the slice contains it"""
                    N = (kq + 1) * 128
                    s_ps = ps_s.tile([128, 1024], FP, tag="s")
                    hp, hb = (h % 2) * 32, h // 2
                    for cc0 in range(c0, c1, 512):
                        cc1 = min(cc0 + 512, c1)
                        nc.tensor.matmul(
                            s_ps[:, cc0 - c0:cc1 - c0],
                            lhsT=qhT[hp:hp + 32, hb, kq * 128:(kq + 1) * 128],
                            rhs=khT[hp:hp + 32, hb, cc0:cc1],
                            start=True, stop=True, skip_group_check=True)
                    if c1 == N:
                        nc.tensor.matmul(s_ps[:, c1 - c0 - 128:c1 - c0],
                                         lhsT=ident_b[:], rhs=triu_neg[:],
                                         start=False, stop=True,
                                         skip_group_check=True)
                    return s_ps

                def subranges(N):
                    return [(c0, min(c0 + 1024, N)) for c0 in range(0, N, 1024)]

                def stage1(kq, _unused=None):
                    """scores -> softmax-#1 numerators -> suffix-sum scans"""
                    N = (kq + 1) * 128
                    es = {}
                    tail4 = att2.tile([128, HG, s_len + 2], BF, tag="tail4")
                    for h in range(HG):
                        for c0, c1 in subranges(N):
                            s_ps = qk_scores(kq, h, c0, c1)
                            if h not in es:
                                e = atte.tile([128, s_len], BF, tag="e",
                                              name=f"e_{h}")
                                es[h] = e
                            nc.scalar.activation(out=es[h][:, c0:c1],
                                                 in_=s_ps[:, 0:c1 - c0],
                                                 func=AF.Exp)
                    for h in range(HG):
                        nc.vector.memset(tail4[:, h, N:N + 1], 0.0)
                        nc.vector.tensor_tensor_scan(
                            out=tail4[:, h, 0:N][:, ::-1],
                            data0=es[h][:, 0:N][:, ::-1],
                            data1=es[h][:, 0:N][:, ::-1], initial=0.0,
                            op0=OP.add, op1=OP.bypass)
                    return tail4

                def stage2(kq, tail4):
                    """ln(tail); u = ln tail + ln pos + ln gamma^2 - ln sigma"""
                    N = (kq + 1) * 128
                    # Ln+Exp share one ACT table set (Sqrt doesn't fit beside
                    # Exp); tail or pos = +0 gives -inf -> dist=0 -> te=1
                    lnt4 = att2.tile([128, HG, s_len + 2], HF, tag="lnt4")
                    nc.scalar.activation(out=lnt4[:, :, 0:N + 1],
                                         in_=tail4[:, :, 0:N + 1], func=AF.Ln)
                    for h in range(HG):
                        ch = att4.tile([128, 1], FP, tag="ch")
                        nc.vector.tensor_scalar(out=ch[:], in0=lnt4[:, h, 0:1],
                                                scalar1=-1.0,
                                                scalar2=lngsq_sb[:, h:h + 1],
                                                op0=OP.mult, op1=OP.add)
                        nc.vector.scalar_tensor_tensor(
                            out=lnt4[:, h, 1:N + 1], in0=lnt4[:, h, 1:N + 1],
                            scalar=ch[:], in1=lnposM[:, 127 + 128 * kq::-1],
                            op0=OP.add, op1=OP.add)
                    return lnt4

                def stage3(kq, lnt4):
                    """dist=exp(0.5u); te=exp(-dist); s2=max(te,1e-5)*s"""
                    N = (kq + 1) * 128
                    nc.scalar.activation(out=lnt4[:, :, 1:N + 1],
                                         in_=lnt4[:, :, 1:N + 1],
                                         func=AF.Exp, scale=0.5)
                    te4 = att1.tile([128, HG, s_len], BF, tag="te4")
                    nc.scalar.activation(out=te4[:, :, 0:N],
                                         in_=lnt4[:, :, 1:N + 1],
                                         func=AF.Exp, scale=-1.0)
                    s2_4 = att2.tile([128, HG, s_len], HF, tag="s2_4")
                    for h in range(HG):
                        for c0, c1 in subranges(N):
                            s_ps2 = qk_scores(kq, h, c0, c1)
                            nc.vector.scalar_tensor_tensor(
                                out=s2_4[:, h, c0:c1], in0=te4[:, h, c0:c1],
                                scalar=1e-5, in1=s_ps2[:, 0:c1 - c0],
                                op0=OP.max, op1=OP.mult)
                    return s2_4

                def stage4(kq, s2_4):
                    """softmax #2 numerator, transpose, AV, normalize"""
                    N = (kq + 1) * 128
                    nb = kq + 1
                    e2_4 = att1.tile([128, HG, s_len], BF, tag="e2_4")
                    nc.scalar.activation(out=e2_4[:, :, 0:N], in_=s2_4[:, :, 0:N],
                                         func=AF.Exp)
                    e2ts = []
                    for h in range(HG):
                        e2t = att2.tile([128, nqb, 128], BF, tag="e2t")
                        nc.sync.dma_start_transpose(out=e2t[:, 0:nb, :],
                                                    in_=e2_4[:, h, 0:N])
                        e2ts.append(e2t)
                    avs = att2.tile([128, HG, 64], FP, tag="avs")
                    for h in range(HG):
                        av = ps_av.tile([128, 64], FP, tag="av")
                        for c in range(nb):
                            nc.tensor.matmul(av[:, 0:33], lhsT=e2ts[h][:, c, :],
                                             rhs=vh1[:, c, h, :],
                                             start=(c == 0), stop=(c == nb - 1))
                        nc.vector.tensor_copy(out=avs[:, h, 0:33], in_=av[:, 0:33])
                    return avs

                def stage5(kq, avs):
                    """normalize by sigma2 + output projection"""
                    concat = att2.tile([128, 128], FP, tag="concat")
                    for h in range(HG):
                        rec2 = att4.tile([128, 1], FP, tag="rec2")
                        nc.vector.reciprocal(out=rec2[:], in_=avs[:, h, 32:33])
                        nc.scalar.activation(
                            out=concat[:, h * 32:(h + 1) * 32],
                            in_=avs[:, h, 0:32], func=AF.Identity, scale=rec2[:])
                    trp = ps_op.tile([128, 128], FP, tag="trop")
                    nc.tensor.transpose(out=trp[:], in_=concat[:],
                                        identity=ident_f[:])
                    concatT = att2.tile([128, 128], FP, tag="concatT")
                    nc.scalar.activation(out=concatT[:], in_=trp[:], func=AF.Copy)
                    op = ps_op.tile([128, 256], FP, tag="trop")
                    nc.tensor.matmul(op[:], lhsT=concatT[:], rhs=wo_sb[:],
                                     start=True, stop=True)
                    ostg = att2.tile([128, 256], FP, tag="ostg")
                    nc.scalar.activation(out=ostg[:], in_=op[:], func=AF.Copy)
                    nc.sync.dma_start(out=out_part[kq * 128:(kq + 1) * 128, :],
                                      in_=ostg[:])

                def emit_v_proj():
                    # deferred: vh isn't needed until stage 4 of wave 0, so
                    # emitting it here overlaps the pipeline ramp
                    xv_sb = attv.tile([128, nech, s_len], BF)
                    nc.sync.dma_start(out=xv_sb[:],
                                      in_=vT.rearrange("(c p) s -> p c s", p=128))
                    wv2_sb = attv.tile([128, nech, D], BF)
                    nc.sync.dma_start(out=wv2_sb[:],
                                      in_=wvT.rearrange("(c p) d -> p c d", p=128))
                    for sb in range(nqb):
                        ps = ps_av.tile([128, 128], FP, tag="av")
                        for c in range(nech):
                            nc.tensor.matmul(ps[:],
                                             lhsT=xv_sb[:, c, sb * 128:(sb + 1) * 128],
                                             rhs=wv2_sb[:, c, :],
                                             start=(c == 0), stop=False)
                        nc.tensor.matmul(ps[:], lhsT=ones1b[:], rhs=bv_sb[:],
                                         start=False, stop=True)
                        for h in range(HG):
                            nc.vector.tensor_copy(out=vh1[:, sb, h, 0:32],
                                                  in_=ps[:, h * 32:(h + 1) * 32])

                stages = (stage1, stage2, stage3, stage4, stage5)
                waves = [0] + list(range(nqb - 1, 0, -1))
                state = {}
                for i in range(len(waves) + len(stages) - 1):
                    for s in range(len(stages) - 1, -1, -1):
                        w = i - s
                        if 0 <= w < len(waves):
                            prev = state.pop((w, s - 1)) if s else None
                            out = stages[s](waves[w], prev)
                            if s < len(stages) - 1:
                                state[(w, s)] = out
                    if i == 0:
                        emit_v_proj()
    return nc


# ---------------------------------------------------------------------------
# host side
# ---------------------------------------------------------------------------

def _softplus(x):
    return np.logaddexp(0.0, x)


def _make_in_maps(q, k, v, Wq, bq, Wk, bk, Wv, bv, Wo, gammas, s_len=S):
    scale = 1.0 / np.sqrt(np.float32(DK))
    g = -_softplus(gammas.reshape(H).astype(np.float64)).astype(np.float32)
    in_maps = []
    for core in range(NCORES):
        b, grp = core // 2, core % 2
        hsel = slice(grp * HG * DK, (grp + 1) * HG * DK)   # rows of W, dims of proj
        gam = g[grp * HG:(grp + 1) * HG]
        in_maps.append({
            "qT": np.ascontiguousarray(q[b].T.astype(ml_dtypes.bfloat16)),
            "kT": np.ascontiguousarray(k[b].T.astype(ml_dtypes.bfloat16)),
            "vT": np.ascontiguousarray(v[b].T.astype(ml_dtypes.bfloat16)),
            "wqT": np.ascontiguousarray(
                (Wq[hsel, :] * scale).T.astype(ml_dtypes.bfloat16)),
            "wkT": np.ascontiguousarray(Wk[hsel, :].T.astype(ml_dtypes.bfloat16)),
            "wvT": np.ascontiguousarray(Wv[hsel, :].T.astype(ml_dtypes.bfloat16)),
            "woT": np.ascontiguousarray(Wo[:, hsel].T.astype(np.float32)),
            "bqs": np.ascontiguousarray(
                (bq[hsel] * scale).astype(np.float32).reshape(2, 64).T),
            "bks": np.ascontiguousarray(
                bk[hsel].astype(np.float32).reshape(2, 64).T),
            "bvrow": bv[hsel].astype(ml_dtypes.bfloat16).reshape(1, D),
            "lngsq": np.broadcast_to(
                (2.0 * np.log(-gam)).astype(np.float32), (128, HG)).copy(),
        })
    return in_maps


_NC_CACHE = {}


def _get_nc(s_len=S):
    if s_len not in _NC_CACHE:
        nc = build_nc(s_len)
        nc.finalize()      # Bacc pipeline: wait splitting, reg alloc, DCE
        _NC_CACHE[s_len] = nc
    return _NC_CACHE[s_len]


def kernel(q, k, v, mask, Wq, bq, Wk, bk, Wv, bv, Wo, bo, gammas):
    """Full-input, full-output entry point.  `mask` is the causal mask the
    reference builds; the kernel hardcodes causality."""
    from concourse.bass_utils import run_bass_kernel_spmd

    q, k, v = (np.asarray(a, np.float32) for a in (q, k, v))
    in_maps = _make_in_maps(q, k, v, np.asarray(Wq), np.asarray(bq),
                            np.asarray(Wk), np.asarray(bk), np.asarray(Wv),
                            np.asarray(bv), np.asarray(Wo),
                            np.asarray(gammas))
    nc = _get_nc(S)
    res = run_bass_kernel_spmd(nc, in_maps, core_ids=list(range(NCORES)))
    parts = [res.results[c]["out_part"] for c in range(NCORES)]
    out = np.empty((B, S, E), np.float32)
    bo = np.asarray(bo, np.float32)
    for b in range(B):
        out[b] = parts[2 * b] + parts[2 * b + 1] + bo[None, :]
    return out
